# revision 11
# baseline (speedup 1.0000x reference)
"""Trainium2 Bass kernel for the OT (Sinkhorn) loss.

Key algebraic structure: the grid is separable, so the 1024x16384 Gibbs
kernel factorizes K[n,(iy,ix)] = A[n,ix]*B[n,iy] with A,B only [1024,128]:
    A[n,ix] = exp(-(px[n]-c[ix])^2/REG),  B[n,iy] = exp(-(py[n]-c[iy])^2/REG)
Each Sinkhorn matvec then becomes real [1024,128]x[128,128] matmuls:
    (u@K)^T[ix,iy] = sum_n A[n,ix] * (B*u)[n,iy]
    (K@v)[n]       = sum_iy B[n,iy] * (A @ V^T)[n,iy]
and the final loss sum(dis * P) reduces to two more matmul groups via
dis[n,m] = Sx[n,ix] + Sy[n,iy].

Distribution: data-parallel over the batch (4 images); core c computes
image c % 4, host sums the 4 scalars. No collectives.

Host prep: A, B' = B/1024, A*Sx, B'*Sy are tiny [1024,128] constants
(~0.007% of total FLOPs) computed host-side with libm exp for accuracy and
shipped as inputs; a = 1/1024 is folded into B' so u' = 1024*u; eps is
injected as a rank-1 (K=1) matmul into the PSUM accumulation; Kv's
*1024 + eps ride along in a fused tensor_tensor_reduce (scale / initial).

Precision: all matmuls fp32 (bf16 operand quantization of the recurrent
u/v state measurably hurts); ~1e-7 relative error on the loss.
"""

import numpy as np
import ml_dtypes

B_IMGS = 4
N_PTS = 1024
G = 128
NT = N_PTS // 128  # 8 n-tiles
REG = 10.0
EPS = 1e-16
N_ITER = 100
N_CORES = 8

_cache = {}


def _ts(t, size=128):
    return slice(t * size, (t + 1) * size)


def _build_program(n_iter=N_ITER):
    import concourse.bacc as bacc
    import concourse.mybir as mybir
    import concourse.tile as tile

    f32 = mybir.dt.float32
    bf16 = mybir.dt.bfloat16
    Copy = mybir.ActivationFunctionType.Copy
    mult = mybir.AluOpType.mult
    add = mybir.AluOpType.add

    nc = bacc.Bacc(
        "TRN2",
        target_bir_lowering=False,
        debug=False,
        enable_asserts=False,
        num_devices=N_CORES,
    )

    ins_spec = [
        ("A32", [128, N_PTS], f32),     # A,   [n-part, t*128+ix]
        ("AT32", [128, N_PTS], f32),    # A^T, [ix-part, t*128+n]
        ("Adx32", [128, N_PTS], f32),   # A*Sx
        ("Bp32", [128, N_PTS], f32),    # B/1024
        ("B32", [128, N_PTS], f32),     # B (unscaled; Kv reduce needs 1024*B')
        ("Bdy32", [128, N_PTS], f32),   # (B/1024)*Sy
        ("densT", [128, 128], f32),     # dens^T, [ix, iy]
        ("onescol", [128, 1], f32),
    ]
    dr = {
        name: nc.dram_tensor(name, shape, dt, kind="ExternalInput").ap()
        for name, shape, dt in ins_spec
    }
    loss_out_d = nc.dram_tensor("loss_out", [1, 1], f32, kind="ExternalOutput").ap()

    with tile.TileContext(nc) as tc:
        with (
            tc.tile_pool(name="const", bufs=1) as cp,
            tc.tile_pool(name="work", bufs=2) as wp,
            tc.tile_pool(name="scr", bufs=1) as sp,
            tc.tile_pool(name="psum", bufs=3, space="PSUM") as ktu_pool,
            tc.tile_pool(name="psumw", bufs=4, space="PSUM") as w_pool,
            tc.tile_pool(name="psuml", bufs=1, space="PSUM") as l_pool,
        ):
            sb = {}
            for name, shape, dt in ins_spec:
                sb[name] = cp.tile(shape, dt, tag=name, name=f"sb_{name}")
                nc.sync.dma_start(out=sb[name][:], in_=dr[name][:])
            A32, AT32, Adx32 = sb["A32"], sb["AT32"], sb["Adx32"]
            Bp32, B32, Bdy32 = sb["Bp32"], sb["B32"], sb["Bdy32"]
            densT, onescol = sb["densT"], sb["onescol"]

            # ---- Sinkhorn loop (fully unrolled) ----
            up = None    # u' = 1024*u, [128, NT] f32 (col t = n-tile t)
            rT = None
            for i in range(n_iter):
                if i == 0:
                    bu = [Bp32[:, _ts(t)] for t in range(NT)]  # u'_0 = 1
                else:
                    bu = []
                    for t in range(NT):
                        bt = wp.tile([128, 128], f32, tag=f"bu{t}",
                                     name=f"bu{t}_{i}", bufs=2)
                        nc.vector.tensor_scalar(
                            out=bt[:], in0=Bp32[:, _ts(t)],
                            scalar1=up[t][:], scalar2=None, op0=mult)
                        bu.append(bt[:])
                # KTu^T[ix,iy] = sum_t A_t^T @ Bu_t  (+eps on DVE)
                ktu = ktu_pool.tile([128, 128], f32, tag="ktu")
                for t in range(NT):
                    nc.tensor.matmul(
                        ktu[:], A32[:, _ts(t)], bu[t],
                        start=(t == 0), stop=(t == NT - 1))
                ke = wp.tile([128, 128], f32, tag="ke")
                nc.vector.tensor_scalar(
                    out=ke[:], in0=ktu[:], scalar1=EPS, scalar2=None, op0=add)
                rT = wp.tile([128, 128], f32, tag="rt")
                nc.vector.reciprocal_approx_fast(out=rT[:], in_=ke[:])
                vT = wp.tile([128, 128], f32, tag="vt")
                nc.vector.tensor_tensor(out=vT[:], in0=rT[:], in1=densT[:], op=mult)
                # Kv_t = sum_iy (AT_t^T @ vT) * B_t ;  u'_t = 1/(Kv_t + eps)
                # per-column so iteration i+1's Bu_t/KTu_t unblock early
                up = []
                for t in range(NT):
                    w = w_pool.tile([128, 128], f32, tag="w")
                    nc.tensor.matmul(w[:], AT32[:, _ts(t)], vT[:], start=True, stop=True)
                    z = wp.tile([128, 128], f32, tag="z")
                    nc.vector.tensor_tensor(
                        out=z[:], in0=w[:], in1=B32[:, _ts(t)], op=mult)
                    kt = wp.tile([128, 1], f32, tag=f"kv{t}",
                                 name=f"kv{t}_{i}", bufs=2)
                    nc.vector.tensor_reduce(
                        out=kt[:], in_=z[:], axis=mybir.AxisListType.X, op=add)
                    ut = wp.tile([128, 1], f32, tag=f"up{t}",
                                 name=f"up{t}_{i}", bufs=2)
                    nc.vector.tensor_scalar(
                        out=kt[:], in0=kt[:], scalar1=EPS, scalar2=None, op0=add)
                    nc.vector.reciprocal_approx_fast(out=ut[:], in_=kt[:])
                    up.append(ut)

            # ---- final loss: sum(dis * P) in fp32 ----
            vT32 = vT  # in-loop vT is already fp32
            M = w_pool.tile([128, 128], f32, tag="w")  # [ix, iy] accumulator
            for t in range(NT):
                buf = wp.tile([128, 128], f32, tag="buf")       # B'_t * u'_t
                nc.vector.tensor_scalar(
                    out=buf[:], in0=Bp32[:, _ts(t)], scalar1=up[t][:],
                    scalar2=None, op0=mult)
                budy = wp.tile([128, 128], f32, tag="budy")     # B'Sy_t * u'_t
                nc.vector.tensor_scalar(
                    out=budy[:], in0=Bdy32[:, _ts(t)], scalar1=up[t][:],
                    scalar2=None, op0=mult)
                # M[ix,iy] += Adx_t^T @ BuF_t + A_t^T @ Budy_t
                nc.tensor.matmul(
                    M[:], Adx32[:, _ts(t)], buf[:], start=(t == 0), stop=False)
                nc.tensor.matmul(
                    M[:], A32[:, _ts(t)], budy[:], start=False, stop=(t == NT - 1))
            zf = sp.tile([128, 128], f32, tag="zf")
            nc.vector.tensor_tensor(out=zf[:], in0=M[:], in1=vT32[:], op=mult)
            s1 = sp.tile([128, 1], f32, tag="s1")
            nc.vector.tensor_reduce(
                out=s1[:], in_=zf[:], axis=mybir.AxisListType.X, op=add)
            loss_ps = l_pool.tile([1, 1], f32, tag="lps")
            nc.tensor.matmul(loss_ps[:], s1[:], onescol[:], start=True, stop=True)
            loss_sb = sp.tile([1, 1], f32, tag="lsb")
            nc.scalar.activation(loss_sb[:], loss_ps[:], Copy)
            nc.sync.dma_start(out=loss_out_d[:], in_=loss_sb[:])

    nc.compile()
    return nc


def _get_program(n_iter=N_ITER):
    key = ("nc", n_iter)
    if key not in _cache:
        _cache[key] = _build_program(n_iter)
    return _cache[key]


def _host_inputs(normed_density, points):
    f32 = np.float32
    coords = (np.arange(0, 1024, 8, dtype=f32) + 4.0).astype(f32)
    onescol = np.ones((128, 1), dtype=f32)
    lnb = f32(np.log(1024.0))

    def tiled(x):  # [1024, 128] -> [128, 8*128] (n-tile layout)
        return np.ascontiguousarray(
            x.reshape(NT, 128, 128).transpose(1, 0, 2).reshape(128, N_PTS))

    in_maps = []
    for c in range(N_CORES):
        img = c % B_IMGS
        px = np.asarray(points[img, :, 0], dtype=f32)
        py = np.asarray(points[img, :, 1], dtype=f32)
        dx = (coords[None, :] - px[:, None]).astype(f32)
        dy = (coords[None, :] - py[:, None]).astype(f32)
        Sx = (dx * dx).astype(f32)
        Sy = (dy * dy).astype(f32)
        A = np.exp((Sx * f32(-1.0 / REG)).astype(f32)).astype(f32)
        Bp = np.exp(((Sy * f32(-1.0 / REG)) - lnb).astype(f32)).astype(f32)
        in_maps.append({
            "A32": tiled(A),
            "AT32": np.ascontiguousarray(A.T),
            "Adx32": tiled((A * Sx).astype(f32)),
            "Bp32": tiled(Bp),
            "B32": tiled(np.exp((Sy * f32(-1.0 / REG)).astype(f32)).astype(f32)),
            "Bdy32": tiled((Bp * Sy).astype(f32)),
            "densT": np.ascontiguousarray(
                np.asarray(normed_density[img, 0], dtype=f32).T),
            "onescol": onescol,
        })
    return in_maps


def _run(normed_density, points, trace=False):
    from concourse.bass_utils import run_bass_kernel_spmd

    nc = _get_program()
    in_maps = _host_inputs(normed_density, points)
    res = run_bass_kernel_spmd(
        nc, in_maps, list(range(N_CORES)), trace=trace)
    per = np.array(
        [res.results[i]["loss_out"][0, 0] for i in range(B_IMGS)],
        dtype=np.float32)
    return per, res


def kernel(normed_density, unnormed_density=None, points=None):
    per, _ = _run(normed_density, points, trace=False)
    total = np.float32(per.sum(dtype=np.float32))
    loss = np.array([total], dtype=np.float32)
    wd = np.float32(total)
    ot_obj_values = np.zeros((1,), dtype=np.float32)
    return (loss, wd, ot_obj_values)
